# revision 8
# baseline (speedup 1.0000x reference)
"""Trainium2 Bass kernel for nn_Net_91164975824989.

Math: the line-MLP consumes binary spike vectors s in {0,1}^3, so
MLP+softmax collapses to an 8-entry LUT; softmax over 2 outputs sums
to 1 => out[:,0] = 150 - out[:,1].  The LUT expands into a multilinear
polynomial over spike bits: per sample we need 33 monomial sums over
the 25 LIF timesteps (9 cell sums, 18 in-line pair products, 6 in-line
triples), then a weighted sum (weights from the tiny MLP, host f64).

Engine split (per core, 4096 samples = 128 partitions x 32/partition):
  - LIF recurrence in zeta-space (zeta = mem - 1): one fused DVE op per
    step: zeta' = beta*zeta + tau - (zeta > 0), tau = x - (1-beta).
    The first step is fused straight from x (LIF_X op); the last step
    fuses its spike compare (LIF_SPK op).
  - Spike recovery OFF the serial chain, on the otherwise-idle Scalar
    engine: Sigmoid(1e30 * zeta) = exact saturated 0/1.
  - Pair products on the Vector engine (bf16, 2x mode); triple products
    and some pair groups on GPSIMD (otherwise idle).
  - Time-accumulation on the TensorEngine: per-weight-class SCALED
    identity matmuls (10 alpha*I matrices) accumulate EVERY feature,
    pre-weighted, into ONE shared [128, 3, 32] PSUM region -- the
    epilogue is a single 96-wide reduce; k1/out0 finalize on host.
  - Pipeline: chunk-i recurrence steps interleave chunk-(i-2) product
    ops so the Vector engine never stalls on recovery latency; cell-sum
    matmuls issue as soon as recovery lands, pair matmuls after their
    products.  TimelineSim-tuned chunk bounds.
"""

import numpy as np

B = 32768
N_CORES = 8
B_CORE = B // N_CORES          # 4096
P = 128                        # partitions
SPP = B_CORE // P              # 32 samples per partition
C = 9                          # cells
T = 25                         # timesteps (t=0 spikes are always 0)
NT = T - 1                     # active timesteps t=1..24 (tau index 0..23)
BETA = 0.95

# chunk bounds over tau=0..23 (spike index); recovery+products pipeline
BOUNDS = (4, 8, 14, 17, 20, 22, 23, 24)
N_WARM = 8                     # dummy warm-up matmuls
WARM_FD = 384

_STATE: dict = {}


def _host_coeffs(W1, b1, W2, b2, W3, b3, W4, b4):
    """8-entry LUT of the line-MLP p1 output -> multilinear coeffs.
    Returns (alphas[10], k1): the 10 weight-class scalars for the scaled
    identity matrices, and the constant term."""
    W1, b1, W2, b2, W3, b3, W4, b4 = [
        np.asarray(a, np.float64) for a in (W1, b1, W2, b2, W3, b3, W4, b4)
    ]

    def mlp_p1(s):
        h = np.maximum(W1 @ s + b1, 0)
        h = np.maximum(W2 @ h + b2, 0)
        h = np.maximum(W3 @ h + b3, 0)
        h = np.maximum(W4 @ h + b4, 0)
        e = np.exp(h - h.max())
        return e[1] / e.sum()

    u = np.zeros(8)
    for code in range(8):
        s = np.array([(code >> j) & 1 for j in range(3)], np.float64)
        u[code] = mlp_p1(s)

    # Moebius transform: u(s) = sum_m c[m] * prod_{j in m} s_j
    c = np.zeros(8)
    for m in range(8):
        for mp in range(8):
            if (mp & m) == mp:
                c[m] += (-1) ** bin(m ^ mp).count("1") * u[mp]

    c_s = [c[1], c[2], c[4]]
    # cell weight classes by sorted (i, j) of cell=3i+j; each cell is in
    # row-line i (position j) and col-line j (position i)
    cellw = {
        (0, 0): c_s[0] + c_s[0], (0, 1): c_s[0] + c_s[1],
        (0, 2): c_s[0] + c_s[2], (1, 1): c_s[1] + c_s[1],
        (1, 2): c_s[1] + c_s[2], (2, 2): c_s[2] + c_s[2],
    }
    alphas = [cellw[(0, 0)], cellw[(0, 1)], cellw[(0, 2)], cellw[(1, 1)],
              cellw[(1, 2)], cellw[(2, 2)], c[3], c[6], c[5], c[7]]
    k1 = 150.0 * c[0]
    return np.array(alphas), k1


def _register_aux_ops():
    """LIF_X_ANT: first step straight from x:
         out = s0*in0 + s1 - (in0 > imm2)    [in1 unused]
       LIF_SPK_ANT: fused last step + spike compare:
         out = (s0*in0 + in1 - (in0 > 0)) > 0
    """
    import re
    from concourse import dve_ops
    from concourse.dve_spec import Spec, Src0, Src1, C0, C1, C2, Zero

    inner = Src0 * C0 + Src1 - (Src0 > Zero)
    out = []
    for name, spec in (
        ("LIF_X_ANT", Spec(
            body=Src0 * C0 + C1 - (Src0 > C2),
            reference=lambda in0, in1, s0, s1, imm2: in0 * s0 + s1
            - (in0 > imm2).astype(in0.dtype))),
        ("LIF_SPK_ANT", Spec(
            body=(Src0 * C0 + Src1 - (Src0 > Zero)) > Zero,
            reference=lambda in0, in1, s0, s1, imm2: (
                (in0 * s0 + in1 - (in0 > 0).astype(in0.dtype)) > 0
            ).astype(in0.dtype))),
        ("LIF_ZETA2_ANT", Spec(
            body=inner * C0 + Src1 - (inner > Zero),
            reference=lambda in0, in1, s0, s1, imm2: (
                lambda z1: z1 * s0 + in1 - (z1 > 0).astype(in0.dtype))(
                in0 * s0 + in1 - (in0 > 0).astype(in0.dtype)))),
    ):
        existing = [o for o in dve_ops.OPS if o.name == name]
        if existing:
            out.append(existing[0])
            continue
        op = dve_ops.DveOp(name, spec, subdim=False, uops_sha={})
        dve_ops.OPS.append(op)
        dve_ops.CUSTOM_DVE_SPECS[op.name] = spec
        dve_ops._SUB_OPCODE_FOR_NAME[op.name] = (
            max(dve_ops._SUB_OPCODE_FOR_NAME.values()) + 1)
        for ver in ("v3", "v4"):
            try:
                op.compile(ver)
            except ValueError as e:
                m = re.search(r'\]="([0-9a-f]+)"', str(e))
                if not m:
                    raise
                op.uops_sha[ver] = m.group(1)
        out.append(op)
    return out


def _register_zeta_op():
    """Custom fused DVE op: out = s0*in0 + in1 - (in0 > 0)  (LIF step in
    zeta-space). Self-pins the uops sha; numerics verified end-to-end."""
    import re
    from concourse import dve_ops
    from concourse.dve_spec import Spec, Src0, Src1, C0, Zero

    for o in dve_ops.OPS:
        if o.name == "LIF_ZETA_ANT":
            return o
    spec = Spec(
        body=Src0 * C0 + Src1 - (Src0 > Zero),
        reference=lambda in0, in1, s0, s1, imm2: in0 * s0 + in1
        - (in0 > 0).astype(in0.dtype),
    )
    op = dve_ops.DveOp("LIF_ZETA_ANT", spec, subdim=False, uops_sha={})
    dve_ops.OPS.append(op)
    dve_ops.CUSTOM_DVE_SPECS[op.name] = spec
    dve_ops._SUB_OPCODE_FOR_NAME[op.name] = (
        max(dve_ops._SUB_OPCODE_FOR_NAME.values()) + 1)
    for ver in ("v3", "v4"):
        try:
            op.compile(ver)
        except ValueError as e:
            m = re.search(r'\]="([0-9a-f]+)"', str(e))
            if not m:
                raise
            op.uops_sha[ver] = m.group(1)
    return op


def _build_module(bounds=BOUNDS, n_warm=N_WARM, y_queue="sp",
                  pool_pairs_chunks=1, late_psb_first=False,
                  x_queue="sp", pool_rp_chunks=0, pbounds=None,
                  prologue="full", pool_tri_tail=2, serial_products=0):
    import concourse.bass as bass
    import concourse.tile as tile
    from concourse import bacc, mybir
    from contextlib import ExitStack

    zeta_op = _register_zeta_op()
    x_op, spk_op, d_op = _register_aux_ops()

    f32 = mybir.dt.float32
    bf16 = mybir.dt.bfloat16
    Alu = mybir.AluOpType
    Act = mybir.ActivationFunctionType

    nc = bacc.Bacc("TRN2", target_bir_lowering=False, debug=False,
                   num_devices=N_CORES)

    # blob per partition: [ 10 scaled identities (128 bf16 = 64 f32 each)
    #                     | k1 | 150-k1 ]
    NID = 10
    BLOB = NID * (P // 2) + 4
    xs = nc.declare_dram_parameter("xs", [B_CORE, C], f32, isOutput=False)
    blob = nc.declare_dram_parameter("blob", [P, BLOB], f32, isOutput=False)
    y = nc.declare_dram_parameter("y", [B_CORE], f32, isOutput=True)

    H = SPP // 2

    with tile.TileContext(nc) as tc, ExitStack() as ctx:
        pool = ctx.enter_context(tc.tile_pool(name="main", bufs=1))
        psum = ctx.enter_context(tc.tile_pool(name="psum", bufs=1, space="PSUM"))

        # ---- ACT table warm-up: dummy sigmoid so the LoadActFuncSet
        # (~1.3us) runs before x arrives instead of delaying recovery ----
        act_warm = pool.tile([P, 2], f32)
        nc.gpsimd.memset(act_warm, 0)
        nc.scalar.activation(act_warm, act_warm, Act.Sigmoid, scale=1.0)

        # ---- input DMAs ----
        x_raw = pool.tile([P, SPP, C], f32)
        xs_r = xs.rearrange("(p s) c -> p s c", p=P)
        (nc.gpsimd if x_queue == "pool" else nc.sync).dma_start(x_raw, xs_r)
        blob_sb = pool.tile([P, BLOB], f32)
        nc.sync.dma_start(blob_sb, blob[:, :])
        ids = blob_sb[:, :NID * (P // 2)].bitcast(bf16).rearrange(
            "p (k q) -> p k q", k=NID)          # [P, 10, 128]
        k1_ap = blob_sb[:, NID * (P // 2):NID * (P // 2) + 1]
        k150_ap = blob_sb[:, NID * (P // 2) + 1:NID * (P // 2) + 2]
        nbeta_ap = blob_sb[:, NID * (P // 2) + 2:NID * (P // 2) + 3]
        nbig_ap = blob_sb[:, NID * (P // 2) + 3:NID * (P // 2) + 4]

        # ---- prologue (halves, so the first half-chain starts asap) ----
        # zh[k] = zeta_{k+2}, k = 0..23; zeta_1 = x-1 never spikes, so
        # zeta_2 = (1+beta)*x - 1.  tau = x - (1-beta) feeds every step.
        tau = pool.tile([P, C, SPP], f32)
        zh = pool.tile([P, NT, C, SPP], f32)
        # tau only; zh[1] comes straight from x (LIF_X op), and the
        # tau=0 spikes come straight from x on ACT.
        if prologue == "tau_act":
            nc.scalar.activation(
                tau.rearrange("p c s -> p s c"), x_raw, Act.Identity,
                bias=nbeta_ap, scale=1.0)
        else:
            nc.vector.tensor_scalar(
                out=tau.rearrange("p c s -> p s c"), in0=x_raw,
                scalar1=-(1.0 - BETA), scalar2=None, op0=Alu.add)

        # spikes sh[tau] = (zeta_{tau+2} > 0), tau = 0..23   (bf16 0/1)
        sh = pool.tile([P, NT, C, SPP], bf16)
        sh_r = sh.rearrange("p t (i j) s -> p t i j s", i=3)

        # product history (bf16)
        rp01 = pool.tile([P, NT, 3, 2, SPP], bf16)
        rp02 = pool.tile([P, NT, 3, SPP], bf16)
        rtr = pool.tile([P, NT, 3, SPP], bf16)
        cp03 = pool.tile([P, NT, 6, SPP], bf16)
        cp06 = pool.tile([P, NT, 3, SPP], bf16)
        ctr = pool.tile([P, NT, 3, SPP], bf16)

        # Single shared PSUM accumulator: every (pre-weighted) matmul
        # accumulates into the same [P, 3, SPP] region -- the final
        # feature reduce is then only 3*SPP wide.
        ps_all = psum.tile([P, 3, SPP], f32)

        # identity index per cell (cell c = 3i+j -> class {i, j})
        IA00, IA01, IA02, IA11, IA12, IA22, IC3, IC6, IC5, IC7 = range(NID)
        CELL_ID = [IA00, IA01, IA02, IA01, IA11, IA12, IA02, IA12, IA22]

        assert bounds[-1] == NT
        nchunks = len(bounds)
        chunks = [(0 if ci == 0 else bounds[ci - 1], b)
                  for ci, b in enumerate(bounds)]

        def recovery(t0, t1, engine):
            if t0 == 0:
                # sh[0] = (zeta_2 > 0) = ((1+beta)x - 1 > 0), from x directly
                nc.scalar.activation(sh[:, 0].rearrange("p c s -> p s c"),
                                     x_raw, Act.Sigmoid,
                                     scale=(1.0 + BETA) * 1e30, bias=nbig_ap)
                t0 = 1
            t1 = min(t1, NT - 1)   # sh[NT-1] comes from the fused step
            if t1 <= t0:
                return
            if engine == "act":
                nc.scalar.activation(sh[:, t0:t1], zh[:, t0:t1],
                                     Act.Sigmoid, scale=1e30)
            else:
                nc.vector.tensor_scalar(out=sh[:, t0:t1], in0=zh[:, t0:t1],
                                        scalar1=0.0, scalar2=None,
                                        op0=Alu.is_gt)

        def product_thunks(t0, t1, pool_triples, pool_pairs=False,
                           pool_rp=False):
            """Deferred product ops for chunk [t0, t1). Triple products go
            to GPSIMD for early chunks (otherwise idle); the last chunks
            keep them on DVE so the PE/epilogue isn't gated on a lagging
            Pool queue."""
            tsl = slice(t0, t1)
            tri = nc.gpsimd if pool_triples else nc.vector
            pr = nc.gpsimd if pool_pairs else nc.vector

            def _rp01():
                nc.vector.tensor_mul(rp01[:, tsl], sh_r[:, tsl, :, 0:2],
                                     sh_r[:, tsl, :, 1:3])
                tri.tensor_mul(rtr[:, tsl], rp01[:, tsl, :, 0],
                               sh_r[:, tsl, :, 2])

            def _cp03():
                nc.vector.tensor_mul(cp03[:, tsl], sh[:, tsl, 0:6],
                                     sh[:, tsl, 3:9])
                tri.tensor_mul(ctr[:, tsl], cp03[:, tsl, 0:3],
                               sh[:, tsl, 6:9])

            pair_thunks = [
                lambda: pr.tensor_mul(rp02[:, tsl],
                                      sh_r[:, tsl, :, 0],
                                      sh_r[:, tsl, :, 2]),
                lambda: pr.tensor_mul(cp06[:, tsl], sh[:, tsl, 0:3],
                                      sh[:, tsl, 6:9]),
            ]
            if pool_pairs:
                # pool queue: pairs (spike-only deps) before triples, so the
                # Pool engine isn't head-blocked on DVE-produced rp01/cp03
                return pair_thunks + [_rp01, _cp03]
            return [_rp01, _cp03] + pair_thunks

        def mms_cells(t0, t1):
            """cell-sum matmuls: depend only on recovery (not products),
            so they're emitted right after the chunk's recovery. The very
            first one carries start=True for the shared psum group."""
            for t in range(t0, t1):
                for c in range(C):
                    nc.tensor.matmul(ps_all[:, 0:1], ids[:, CELL_ID[c]],
                                     sh[:, t, c:c + 1],
                                     start=(t == 0 and c == 0), stop=False,
                                     skip_group_check=True)

        def mms_pairs(t0, t1):
            for t in range(t0, t1):
                sp = t == NT - 1
                items = [(IC3, rp01[:, t, :, 0]),
                         (IC6, rp01[:, t, :, 1]),
                         (IC5, rp02[:, t]),
                         (IC7, rtr[:, t]),
                         (IC3, cp03[:, t, 0:3]),
                         (IC6, cp03[:, t, 3:6]),
                         (IC5, cp06[:, t]),
                         (IC7, ctr[:, t])]
                for i, (idk, rhs) in enumerate(items):
                    nc.tensor.matmul(ps_all[:, 0:3], ids[:, idk], rhs,
                                     start=False,
                                     stop=sp and i == len(items) - 1,
                                     skip_group_check=True)

        # ---- main pipeline ----
        # DVE stream: chunk-i steps interleave chunk-(i-2) product ops
        # (2-chunk lag hides the ACT recovery latency). First two chunks
        # have no partner work: emit steps as halves to hide pipe drain.
        xr_perm = x_raw.rearrange("p s c -> p c s")

        def chain_step(k):
            """advance the serial chain along ODD zh indices (2 LIF steps
            per fused op); k == NT-1 fuses the final spike compare."""
            if k == 1:
                # zeta_3 = beta*zeta_2 + tau - (zeta_2 > 0) with
                # zeta_2 = (1+beta)x - 1: expanded as a pure function of x
                nc.vector._custom_dve(
                    x_op, out=zh[:, 1], in0=xr_perm,
                    s0=BETA * (1.0 + BETA) + 1.0,
                    s1=-(BETA + (1.0 - BETA)),
                    imm2=1.0 / (1.0 + BETA))
            elif k == NT - 1:
                nc.vector._custom_dve(spk_op, out=sh[:, NT - 1],
                                      in0=zh[:, k - 1], in1=tau, s0=BETA)
            else:
                nc.vector._custom_dve(d_op, out=zh[:, k], in0=zh[:, k - 2],
                                      in1=tau, s0=BETA)

        def even_fill(evens):
            """reconstruct even zh states from the odd chain, batched."""
            if not evens:
                return
            e0, e1 = evens[0], evens[-1]
            n = len(evens)
            tau_b = tau.rearrange("p (o c) s -> p o (c s)", o=1).broadcast_to(
                (P, n, C * SPP))
            nc.vector._custom_dve(
                zeta_op,
                out=zh[:, e0:e1 + 1:2].rearrange("p t c s -> p t (c s)"),
                in0=zh[:, e0 - 1:e1:2].rearrange("p t c s -> p t (c s)"),
                in1=tau_b, s0=BETA)

        # Product ranges may merge several recovery chunks (fewer, larger
        # DVE ops -> fewer fixed-cost bubbles). Each range becomes ready
        # when its covering recovery chunks have been emitted; its ops are
        # interleaved into steps two chunks later.
        from collections import deque
        pb = list(pbounds) if pbounds is not None else list(bounds)
        assert pb[-1] == NT and set(pb) <= set(bounds)
        pranges = [(0 if i == 0 else pb[i - 1], b) for i, b in enumerate(pb)]
        pending: deque = deque()   # (ready_chunk_idx, thunk)

        if serial_products:
            # Front-loaded chain: emit ALL recurrence work first (chain ops
            # back-to-back with even-fills absorbing the pipe drains), then
            # the whole product phase as pure backlog.  The serial chain
            # finishes ~8us earlier; products become busy-bound.
            for ci, (t0, t1) in enumerate(chunks):
                chain_ks = [k for k in range(max(t0, 1), t1) if k % 2 == 1]
                evens = [k for k in range(max(t0, 2), min(t1, NT - 1))
                         if k % 2 == 0]
                for j, k in enumerate(chain_ks):
                    chain_step(k)
                    if j == 0:
                        even_fill(evens)
                if not chain_ks:
                    even_fill(evens)
                recovery(t0, t1, "act")
                mms_cells(t0, t1)
            for ri, (r0, r1) in enumerate(pranges):
                for th in product_thunks(
                        r0, r1, pool_triples=(ri < serial_products),
                        pool_pairs=(ri < pool_pairs_chunks),
                        pool_rp=(ri < pool_rp_chunks)):
                    th()
                mms_pairs(r0, r1)
        else:
            for ci, (t0, t1) in enumerate(chunks):
                chain_ks = [k for k in range(max(t0, 1), t1) if k % 2 == 1]
                evens = [k for k in range(max(t0, 2), min(t1, NT - 1))
                         if k % 2 == 0]
                for j, k in enumerate(chain_ks):
                    chain_step(k)
                    if j == 0:
                        even_fill(evens)
                    if pending and pending[0][0] <= ci - 2:
                        pending.popleft()[1]()
                if not chain_ks:
                    even_fill(evens)
                while pending and pending[0][0] <= ci - 2:
                    pending.popleft()[1]()
                recovery(t0, t1, "dve" if ci == nchunks - 1 else "act")
                mms_cells(t0, t1)
                for ri, (r0, r1) in enumerate(pranges):
                    if r1 == t1:
                        for th in product_thunks(
                                r0, r1,
                                pool_triples=(ri < len(pranges)
                                              - pool_tri_tail),
                                pool_pairs=(ri < pool_pairs_chunks),
                                pool_rp=(ri < pool_rp_chunks)):
                            pending.append((ci, th))
                        pending.append((ci, lambda r0=r0, r1=r1:
                                        mms_pairs(r0, r1)))
            while pending:
                pending.popleft()[1]()

        # ---- epilogue: one tiny reduce; k1/out0 finalized on host ----
        red = pool.tile([P, SPP], f32)
        nc.vector.tensor_reduce(out=red,
                                in_=ps_all.rearrange("p f s -> p s f"),
                                axis=mybir.AxisListType.X, op=Alu.add)
        yq = nc.gpsimd if y_queue == "pool" else nc.sync
        yq.dma_start(y.rearrange("(p s) -> p s", p=P), red)

    nc.compile()
    return nc


def _get_module():
    if "nc" not in _STATE:
        _STATE["nc"] = _build_module()
    return _STATE["nc"]


def kernel(x, W1, b1, W2, b2, W3, b3, W4, b4, _trace=False):
    import ml_dtypes
    from concourse.bass_utils import run_bass_kernel_spmd

    alphas, k1 = _host_coeffs(W1, b1, W2, b2, W3, b3, W4, b4)

    xs = np.asarray(x, np.float32).reshape(N_CORES, B_CORE, C)
    eye = np.eye(P, dtype=np.float64)
    ids = np.concatenate(
        [np.ascontiguousarray((eye * a).astype(ml_dtypes.bfloat16)).view(
            np.float32) for a in alphas], axis=1)          # [P, 10*64]
    consts = np.tile(
        np.array([[k1, 150.0 - k1, -(1.0 - BETA), -1e30]], np.float32),
        (P, 1))
    blob = np.ascontiguousarray(
        np.concatenate([ids.astype(np.float32), consts], axis=1))

    nc = _get_module()
    in_maps = [{"xs": np.ascontiguousarray(xs[i]), "blob": blob}
               for i in range(N_CORES)]
    res = run_bass_kernel_spmd(nc, in_maps, core_ids=list(range(N_CORES)),
                               trace=_trace)
    red = np.concatenate([res.results[i]["y"] for i in range(N_CORES)])
    out1 = (red.astype(np.float64) + k1).astype(np.float32)
    out = np.stack([150.0 - out1, out1], axis=1).astype(np.float32)
    if _trace:
        _STATE["last_results"] = res
    return out


# revision 9
# speedup vs baseline: 1.0071x; 1.0071x over previous
"""Trainium2 Bass kernel for nn_Net_91164975824989.

Math: the line-MLP consumes binary spike vectors s in {0,1}^3, so
MLP+softmax collapses to an 8-entry LUT; softmax over 2 outputs sums
to 1 => out[:,0] = 150 - out[:,1].  The LUT expands into a multilinear
polynomial over spike bits: per sample we need 33 monomial sums over
the 25 LIF timesteps (9 cell sums, 18 in-line pair products, 6 in-line
triples), then a weighted sum (weights from the tiny MLP, host f64).

Engine split (per core, 4096 samples = 128 partitions x 32/partition):
  - LIF recurrence in zeta-space (zeta = mem - 1): one fused DVE op per
    step: zeta' = beta*zeta + tau - (zeta > 0), tau = x - (1-beta).
    The first step is fused straight from x (LIF_X op); the last step
    fuses its spike compare (LIF_SPK op).
  - Spike recovery OFF the serial chain, on the otherwise-idle Scalar
    engine: Sigmoid(1e30 * zeta) = exact saturated 0/1.
  - Pair products on the Vector engine (bf16, 2x mode); triple products
    and some pair groups on GPSIMD (otherwise idle).
  - Time-accumulation on the TensorEngine: per-weight-class SCALED
    identity matmuls (10 alpha*I matrices) accumulate EVERY feature,
    pre-weighted, into ONE shared [128, 3, 32] PSUM region -- the
    epilogue is a single 96-wide reduce; k1/out0 finalize on host.
  - Pipeline: chunk-i recurrence steps interleave chunk-(i-2) product
    ops so the Vector engine never stalls on recovery latency; cell-sum
    matmuls issue as soon as recovery lands, pair matmuls after their
    products.  TimelineSim-tuned chunk bounds.
"""

import numpy as np

B = 32768
N_CORES = 8
B_CORE = B // N_CORES          # 4096
P = 128                        # partitions
SPP = B_CORE // P              # 32 samples per partition
C = 9                          # cells
T = 25                         # timesteps (t=0 spikes are always 0)
NT = T - 1                     # active timesteps t=1..24 (tau index 0..23)
BETA = 0.95

# chunk bounds over tau=0..23 (spike index); recovery+products pipeline
BOUNDS = (4, 8, 14, 17, 20, 22, 23, 24)
N_WARM = 8                     # dummy warm-up matmuls
WARM_FD = 384

_STATE: dict = {}


def _host_coeffs(W1, b1, W2, b2, W3, b3, W4, b4):
    """8-entry LUT of the line-MLP p1 output -> multilinear coeffs.
    Returns (alphas[10], k1): the 10 weight-class scalars for the scaled
    identity matrices, and the constant term."""
    W1, b1, W2, b2, W3, b3, W4, b4 = [
        np.asarray(a, np.float64) for a in (W1, b1, W2, b2, W3, b3, W4, b4)
    ]

    def mlp_p1(s):
        h = np.maximum(W1 @ s + b1, 0)
        h = np.maximum(W2 @ h + b2, 0)
        h = np.maximum(W3 @ h + b3, 0)
        h = np.maximum(W4 @ h + b4, 0)
        e = np.exp(h - h.max())
        return e[1] / e.sum()

    u = np.zeros(8)
    for code in range(8):
        s = np.array([(code >> j) & 1 for j in range(3)], np.float64)
        u[code] = mlp_p1(s)

    # Moebius transform: u(s) = sum_m c[m] * prod_{j in m} s_j
    c = np.zeros(8)
    for m in range(8):
        for mp in range(8):
            if (mp & m) == mp:
                c[m] += (-1) ** bin(m ^ mp).count("1") * u[mp]

    c_s = [c[1], c[2], c[4]]
    # cell weight classes by sorted (i, j) of cell=3i+j; each cell is in
    # row-line i (position j) and col-line j (position i)
    cellw = {
        (0, 0): c_s[0] + c_s[0], (0, 1): c_s[0] + c_s[1],
        (0, 2): c_s[0] + c_s[2], (1, 1): c_s[1] + c_s[1],
        (1, 2): c_s[1] + c_s[2], (2, 2): c_s[2] + c_s[2],
    }
    alphas = [cellw[(0, 0)], cellw[(0, 1)], cellw[(0, 2)], cellw[(1, 1)],
              cellw[(1, 2)], cellw[(2, 2)], c[3], c[6], c[5], c[7]]
    k1 = 150.0 * c[0]
    return np.array(alphas), k1


def _register_aux_ops():
    """LIF_X_ANT: first step straight from x:
         out = s0*in0 + s1 - (in0 > imm2)    [in1 unused]
       LIF_SPK_ANT: fused last step + spike compare:
         out = (s0*in0 + in1 - (in0 > 0)) > 0
    """
    import re
    from concourse import dve_ops
    from concourse.dve_spec import Spec, Src0, Src1, C0, C1, C2, Zero

    inner = Src0 * C0 + Src1 - (Src0 > Zero)
    out = []
    for name, spec in (
        ("LIF_X_ANT", Spec(
            body=Src0 * C0 + C1 - (Src0 > C2),
            reference=lambda in0, in1, s0, s1, imm2: in0 * s0 + s1
            - (in0 > imm2).astype(in0.dtype))),
        ("LIF_SPK_ANT", Spec(
            body=(Src0 * C0 + Src1 - (Src0 > Zero)) > Zero,
            reference=lambda in0, in1, s0, s1, imm2: (
                (in0 * s0 + in1 - (in0 > 0).astype(in0.dtype)) > 0
            ).astype(in0.dtype))),
        ("LIF_ZETA2_ANT", Spec(
            body=inner * C0 + Src1 - (inner > Zero),
            reference=lambda in0, in1, s0, s1, imm2: (
                lambda z1: z1 * s0 + in1 - (z1 > 0).astype(in0.dtype))(
                in0 * s0 + in1 - (in0 > 0).astype(in0.dtype)))),
    ):
        existing = [o for o in dve_ops.OPS if o.name == name]
        if existing:
            out.append(existing[0])
            continue
        op = dve_ops.DveOp(name, spec, subdim=False, uops_sha={})
        dve_ops.OPS.append(op)
        dve_ops.CUSTOM_DVE_SPECS[op.name] = spec
        dve_ops._SUB_OPCODE_FOR_NAME[op.name] = (
            max(dve_ops._SUB_OPCODE_FOR_NAME.values()) + 1)
        for ver in ("v3", "v4"):
            try:
                op.compile(ver)
            except ValueError as e:
                m = re.search(r'\]="([0-9a-f]+)"', str(e))
                if not m:
                    raise
                op.uops_sha[ver] = m.group(1)
        out.append(op)
    return out


def _register_zeta_op():
    """Custom fused DVE op: out = s0*in0 + in1 - (in0 > 0)  (LIF step in
    zeta-space). Self-pins the uops sha; numerics verified end-to-end."""
    import re
    from concourse import dve_ops
    from concourse.dve_spec import Spec, Src0, Src1, C0, Zero

    for o in dve_ops.OPS:
        if o.name == "LIF_ZETA_ANT":
            return o
    spec = Spec(
        body=Src0 * C0 + Src1 - (Src0 > Zero),
        reference=lambda in0, in1, s0, s1, imm2: in0 * s0 + in1
        - (in0 > 0).astype(in0.dtype),
    )
    op = dve_ops.DveOp("LIF_ZETA_ANT", spec, subdim=False, uops_sha={})
    dve_ops.OPS.append(op)
    dve_ops.CUSTOM_DVE_SPECS[op.name] = spec
    dve_ops._SUB_OPCODE_FOR_NAME[op.name] = (
        max(dve_ops._SUB_OPCODE_FOR_NAME.values()) + 1)
    for ver in ("v3", "v4"):
        try:
            op.compile(ver)
        except ValueError as e:
            m = re.search(r'\]="([0-9a-f]+)"', str(e))
            if not m:
                raise
            op.uops_sha[ver] = m.group(1)
    return op


def _build_module(bounds=BOUNDS, n_warm=N_WARM, y_queue="sp",
                  pool_pairs_chunks=1, late_psb_first=False,
                  x_queue="sp", pool_rp_chunks=0, pbounds=None,
                  prologue="full", pool_tri_tail=2, serial_products=0,
                  pool_fill_chunks=0):
    import concourse.bass as bass
    import concourse.tile as tile
    from concourse import bacc, mybir
    from contextlib import ExitStack

    zeta_op = _register_zeta_op()
    x_op, spk_op, d_op = _register_aux_ops()

    f32 = mybir.dt.float32
    bf16 = mybir.dt.bfloat16
    Alu = mybir.AluOpType
    Act = mybir.ActivationFunctionType

    nc = bacc.Bacc("TRN2", target_bir_lowering=False, debug=False,
                   num_devices=N_CORES)

    # blob per partition: [ 10 scaled identities (128 bf16 = 64 f32 each)
    #                     | k1 | 150-k1 ]
    NID = 10
    BLOB = NID * (P // 2) + 4
    xs = nc.declare_dram_parameter("xs", [B_CORE, C], f32, isOutput=False)
    blob = nc.declare_dram_parameter("blob", [P, BLOB], f32, isOutput=False)
    y = nc.declare_dram_parameter("y", [B_CORE], f32, isOutput=True)

    H = SPP // 2

    with tile.TileContext(nc) as tc, ExitStack() as ctx:
        pool = ctx.enter_context(tc.tile_pool(name="main", bufs=1))
        psum = ctx.enter_context(tc.tile_pool(name="psum", bufs=1, space="PSUM"))

        # ---- ACT table warm-up: dummy sigmoid so the LoadActFuncSet
        # (~1.3us) runs before x arrives instead of delaying recovery ----
        act_warm = pool.tile([P, 2], f32)
        nc.gpsimd.memset(act_warm, 0)
        nc.scalar.activation(act_warm, act_warm, Act.Sigmoid, scale=1.0)

        # ---- input DMAs ----
        x_raw = pool.tile([P, SPP, C], f32)
        xs_r = xs.rearrange("(p s) c -> p s c", p=P)
        (nc.gpsimd if x_queue == "pool" else nc.sync).dma_start(x_raw, xs_r)
        blob_sb = pool.tile([P, BLOB], f32)
        nc.sync.dma_start(blob_sb, blob[:, :])
        ids = blob_sb[:, :NID * (P // 2)].bitcast(bf16).rearrange(
            "p (k q) -> p k q", k=NID)          # [P, 10, 128]
        k1_ap = blob_sb[:, NID * (P // 2):NID * (P // 2) + 1]
        k150_ap = blob_sb[:, NID * (P // 2) + 1:NID * (P // 2) + 2]
        nbeta_ap = blob_sb[:, NID * (P // 2) + 2:NID * (P // 2) + 3]
        nbig_ap = blob_sb[:, NID * (P // 2) + 3:NID * (P // 2) + 4]

        # ---- prologue (halves, so the first half-chain starts asap) ----
        # zh[k] = zeta_{k+2}, k = 0..23; zeta_1 = x-1 never spikes, so
        # zeta_2 = (1+beta)*x - 1.  tau = x - (1-beta) feeds every step.
        tau = pool.tile([P, C, SPP], f32)
        zh = pool.tile([P, NT, C, SPP], f32)
        # tau only; zh[1] comes straight from x (LIF_X op), and the
        # tau=0 spikes come straight from x on ACT.
        if prologue == "tau_act":
            nc.scalar.activation(
                tau.rearrange("p c s -> p s c"), x_raw, Act.Identity,
                bias=nbeta_ap, scale=1.0)
        else:
            nc.vector.tensor_scalar(
                out=tau.rearrange("p c s -> p s c"), in0=x_raw,
                scalar1=-(1.0 - BETA), scalar2=None, op0=Alu.add)

        # spikes sh[tau] = (zeta_{tau+2} > 0), tau = 0..23   (bf16 0/1)
        sh = pool.tile([P, NT, C, SPP], bf16)
        sh_r = sh.rearrange("p t (i j) s -> p t i j s", i=3)

        # product history (bf16)
        rp01 = pool.tile([P, NT, 3, 2, SPP], bf16)
        rp02 = pool.tile([P, NT, 3, SPP], bf16)
        rtr = pool.tile([P, NT, 3, SPP], bf16)
        cp03 = pool.tile([P, NT, 6, SPP], bf16)
        cp06 = pool.tile([P, NT, 3, SPP], bf16)
        ctr = pool.tile([P, NT, 3, SPP], bf16)

        # Single shared PSUM accumulator: every (pre-weighted) matmul
        # accumulates into the same [P, 3, SPP] region -- the final
        # feature reduce is then only 3*SPP wide.
        ps_all = psum.tile([P, 3, SPP], f32)

        # identity index per cell (cell c = 3i+j -> class {i, j})
        IA00, IA01, IA02, IA11, IA12, IA22, IC3, IC6, IC5, IC7 = range(NID)
        CELL_ID = [IA00, IA01, IA02, IA01, IA11, IA12, IA02, IA12, IA22]

        assert bounds[-1] == NT
        nchunks = len(bounds)
        chunks = [(0 if ci == 0 else bounds[ci - 1], b)
                  for ci, b in enumerate(bounds)]

        def recovery(t0, t1, engine):
            if t0 == 0:
                # sh[0] = (zeta_2 > 0) = ((1+beta)x - 1 > 0), from x directly
                nc.scalar.activation(sh[:, 0].rearrange("p c s -> p s c"),
                                     x_raw, Act.Sigmoid,
                                     scale=(1.0 + BETA) * 1e30, bias=nbig_ap)
                t0 = 1
            t1 = min(t1, NT - 1)   # sh[NT-1] comes from the fused step
            if t1 <= t0:
                return
            if engine == "act":
                nc.scalar.activation(sh[:, t0:t1], zh[:, t0:t1],
                                     Act.Sigmoid, scale=1e30)
            else:
                nc.vector.tensor_scalar(out=sh[:, t0:t1], in0=zh[:, t0:t1],
                                        scalar1=0.0, scalar2=None,
                                        op0=Alu.is_gt)

        def product_thunks(t0, t1, pool_triples, pool_pairs=False,
                           pool_rp=False):
            """Deferred product ops for chunk [t0, t1). Triple products go
            to GPSIMD for early chunks (otherwise idle); the last chunks
            keep them on DVE so the PE/epilogue isn't gated on a lagging
            Pool queue."""
            tsl = slice(t0, t1)
            tri = nc.gpsimd if pool_triples else nc.vector
            pr = nc.gpsimd if pool_pairs else nc.vector

            def _rp01():
                nc.vector.tensor_mul(rp01[:, tsl], sh_r[:, tsl, :, 0:2],
                                     sh_r[:, tsl, :, 1:3])
                tri.tensor_mul(rtr[:, tsl], rp01[:, tsl, :, 0],
                               sh_r[:, tsl, :, 2])

            def _cp03():
                nc.vector.tensor_mul(cp03[:, tsl], sh[:, tsl, 0:6],
                                     sh[:, tsl, 3:9])
                tri.tensor_mul(ctr[:, tsl], cp03[:, tsl, 0:3],
                               sh[:, tsl, 6:9])

            pair_thunks = [
                lambda: pr.tensor_mul(rp02[:, tsl],
                                      sh_r[:, tsl, :, 0],
                                      sh_r[:, tsl, :, 2]),
                lambda: pr.tensor_mul(cp06[:, tsl], sh[:, tsl, 0:3],
                                      sh[:, tsl, 6:9]),
            ]
            if pool_pairs:
                # pool queue: pairs (spike-only deps) before triples, so the
                # Pool engine isn't head-blocked on DVE-produced rp01/cp03
                return pair_thunks + [_rp01, _cp03]
            return [_rp01, _cp03] + pair_thunks

        def mms_cells(t0, t1):
            """cell-sum matmuls: depend only on recovery (not products),
            so they're emitted right after the chunk's recovery. The very
            first one carries start=True for the shared psum group."""
            for t in range(t0, t1):
                for c in range(C):
                    nc.tensor.matmul(ps_all[:, 0:1], ids[:, CELL_ID[c]],
                                     sh[:, t, c:c + 1],
                                     start=(t == 0 and c == 0), stop=False,
                                     skip_group_check=True)

        def mms_pairs(t0, t1):
            for t in range(t0, t1):
                sp = t == NT - 1
                items = [(IC3, rp01[:, t, :, 0]),
                         (IC6, rp01[:, t, :, 1]),
                         (IC5, rp02[:, t]),
                         (IC7, rtr[:, t]),
                         (IC3, cp03[:, t, 0:3]),
                         (IC6, cp03[:, t, 3:6]),
                         (IC5, cp06[:, t]),
                         (IC7, ctr[:, t])]
                for i, (idk, rhs) in enumerate(items):
                    nc.tensor.matmul(ps_all[:, 0:3], ids[:, idk], rhs,
                                     start=False,
                                     stop=sp and i == len(items) - 1,
                                     skip_group_check=True)

        # ---- main pipeline ----
        # DVE stream: chunk-i steps interleave chunk-(i-2) product ops
        # (2-chunk lag hides the ACT recovery latency). First two chunks
        # have no partner work: emit steps as halves to hide pipe drain.
        xr_perm = x_raw.rearrange("p s c -> p c s")

        def chain_step(k):
            """advance the serial chain along ODD zh indices (2 LIF steps
            per fused op); k == NT-1 fuses the final spike compare."""
            if k == 1:
                # zeta_3 = beta*zeta_2 + tau - (zeta_2 > 0) with
                # zeta_2 = (1+beta)x - 1: expanded as a pure function of x
                nc.vector._custom_dve(
                    x_op, out=zh[:, 1], in0=xr_perm,
                    s0=BETA * (1.0 + BETA) + 1.0,
                    s1=-(BETA + (1.0 - BETA)),
                    imm2=1.0 / (1.0 + BETA))
            elif k == NT - 1:
                nc.vector._custom_dve(spk_op, out=sh[:, NT - 1],
                                      in0=zh[:, k - 1], in1=tau, s0=BETA)
            else:
                nc.vector._custom_dve(d_op, out=zh[:, k], in0=zh[:, k - 2],
                                      in1=tau, s0=BETA)

        fill_scratch = pool.tile([P, 2, C, SPP], f32)

        def even_fill_pool(evens):
            """same reconstruction via 3 stock GPSIMD ops -- runs in the
            Pool engine's otherwise-dead start window."""
            e0, e1 = evens[0], evens[-1]
            n = len(evens)
            g = fill_scratch[:, 0:n] if n <= 2 else None
            m = fill_scratch[:, 0:n]
            src_ap = zh[:, e0 - 1:e1:2]
            nc.gpsimd.tensor_scalar(out=fill_scratch[:, 0:n], in0=src_ap,
                                    scalar1=0.0, scalar2=None, op0=Alu.is_gt)
            nc.gpsimd.scalar_tensor_tensor(
                out=zh[:, e0:e1 + 1:2], in0=src_ap, scalar=BETA,
                in1=fill_scratch[:, 0:n], op0=Alu.mult, op1=Alu.subtract)
            tau_b = tau.rearrange("p (o c) s -> p o c s", o=1).broadcast_to(
                (P, n, C, SPP))
            nc.gpsimd.tensor_tensor(out=zh[:, e0:e1 + 1:2],
                                    in0=zh[:, e0:e1 + 1:2], in1=tau_b,
                                    op=Alu.add)

        def even_fill(evens):
            """reconstruct even zh states from the odd chain, batched."""
            if not evens:
                return
            e0, e1 = evens[0], evens[-1]
            n = len(evens)
            tau_b = tau.rearrange("p (o c) s -> p o (c s)", o=1).broadcast_to(
                (P, n, C * SPP))
            nc.vector._custom_dve(
                zeta_op,
                out=zh[:, e0:e1 + 1:2].rearrange("p t c s -> p t (c s)"),
                in0=zh[:, e0 - 1:e1:2].rearrange("p t c s -> p t (c s)"),
                in1=tau_b, s0=BETA)

        # Product ranges may merge several recovery chunks (fewer, larger
        # DVE ops -> fewer fixed-cost bubbles). Each range becomes ready
        # when its covering recovery chunks have been emitted; its ops are
        # interleaved into steps two chunks later.
        from collections import deque
        pb = list(pbounds) if pbounds is not None else list(bounds)
        assert pb[-1] == NT and set(pb) <= set(bounds)
        pranges = [(0 if i == 0 else pb[i - 1], b) for i, b in enumerate(pb)]
        pending: deque = deque()   # (ready_chunk_idx, thunk)

        if serial_products:
            # Front-loaded chain: emit ALL recurrence work first (chain ops
            # back-to-back with even-fills absorbing the pipe drains), then
            # the whole product phase as pure backlog.  The serial chain
            # finishes ~8us earlier; products become busy-bound.
            for ci, (t0, t1) in enumerate(chunks):
                chain_ks = [k for k in range(max(t0, 1), t1) if k % 2 == 1]
                evens = [k for k in range(max(t0, 2), min(t1, NT - 1))
                         if k % 2 == 0]
                for j, k in enumerate(chain_ks):
                    chain_step(k)
                    if j == 0:
                        even_fill(evens)
                if not chain_ks:
                    even_fill(evens)
                recovery(t0, t1, "act")
                mms_cells(t0, t1)
            for ri, (r0, r1) in enumerate(pranges):
                for th in product_thunks(
                        r0, r1, pool_triples=(ri < serial_products),
                        pool_pairs=(ri < pool_pairs_chunks),
                        pool_rp=(ri < pool_rp_chunks)):
                    th()
                mms_pairs(r0, r1)
        else:
            for ci, (t0, t1) in enumerate(chunks):
                chain_ks = [k for k in range(max(t0, 1), t1) if k % 2 == 1]
                evens = [k for k in range(max(t0, 2), min(t1, NT - 1))
                         if k % 2 == 0]
                fill = (even_fill_pool if ci < pool_fill_chunks
                        else even_fill)
                for j, k in enumerate(chain_ks):
                    chain_step(k)
                    if j == 0:
                        fill(evens)
                    if pending and pending[0][0] <= ci - 2:
                        pending.popleft()[1]()
                if not chain_ks:
                    fill(evens)
                while pending and pending[0][0] <= ci - 2:
                    pending.popleft()[1]()
                recovery(t0, t1, "dve" if ci == nchunks - 1 else "act")
                mms_cells(t0, t1)
                for ri, (r0, r1) in enumerate(pranges):
                    if r1 == t1:
                        for th in product_thunks(
                                r0, r1,
                                pool_triples=(ri < len(pranges)
                                              - pool_tri_tail),
                                pool_pairs=(ri < pool_pairs_chunks),
                                pool_rp=(ri < pool_rp_chunks)):
                            pending.append((ci, th))
                        pending.append((ci, lambda r0=r0, r1=r1:
                                        mms_pairs(r0, r1)))
            while pending:
                pending.popleft()[1]()

        # ---- epilogue: one tiny reduce; k1/out0 finalized on host ----
        red = pool.tile([P, SPP], f32)
        nc.vector.tensor_reduce(out=red,
                                in_=ps_all.rearrange("p f s -> p s f"),
                                axis=mybir.AxisListType.X, op=Alu.add)
        yq = nc.gpsimd if y_queue == "pool" else nc.sync
        yq.dma_start(y.rearrange("(p s) -> p s", p=P), red)

    nc.compile()
    return nc


def _get_module():
    if "nc" not in _STATE:
        _STATE["nc"] = _build_module()
    return _STATE["nc"]


def kernel(x, W1, b1, W2, b2, W3, b3, W4, b4, _trace=False):
    import ml_dtypes
    from concourse.bass_utils import run_bass_kernel_spmd

    alphas, k1 = _host_coeffs(W1, b1, W2, b2, W3, b3, W4, b4)

    xs = np.asarray(x, np.float32).reshape(N_CORES, B_CORE, C)
    eye = np.eye(P, dtype=np.float64)
    ids = np.concatenate(
        [np.ascontiguousarray((eye * a).astype(ml_dtypes.bfloat16)).view(
            np.float32) for a in alphas], axis=1)          # [P, 10*64]
    consts = np.tile(
        np.array([[k1, 150.0 - k1, -(1.0 - BETA), -1e30]], np.float32),
        (P, 1))
    blob = np.ascontiguousarray(
        np.concatenate([ids.astype(np.float32), consts], axis=1))

    nc = _get_module()
    in_maps = [{"xs": np.ascontiguousarray(xs[i]), "blob": blob}
               for i in range(N_CORES)]
    res = run_bass_kernel_spmd(nc, in_maps, core_ids=list(range(N_CORES)),
                               trace=_trace)
    red = np.concatenate([res.results[i]["y"] for i in range(N_CORES)])
    out1 = (red.astype(np.float64) + k1).astype(np.float32)
    out = np.stack([150.0 - out1, out1], axis=1).astype(np.float32)
    if _trace:
        _STATE["last_results"] = res
    return out


# revision 10
# speedup vs baseline: 1.0105x; 1.0034x over previous
"""Trainium2 Bass kernel for nn_Net_91164975824989.

Math: the line-MLP consumes binary spike vectors s in {0,1}^3, so
MLP+softmax collapses to an 8-entry LUT; softmax over 2 outputs sums
to 1 => out[:,0] = 150 - out[:,1].  The LUT expands into a multilinear
polynomial over spike bits: per sample we need 33 monomial sums over
the 25 LIF timesteps (9 cell sums, 18 in-line pair products, 6 in-line
triples), then a weighted sum (weights from the tiny MLP, host f64).

Engine split (per core, 4096 samples = 128 partitions x 32/partition):
  - LIF recurrence in zeta-space (zeta = mem - 1): one fused DVE op per
    step: zeta' = beta*zeta + tau - (zeta > 0), tau = x - (1-beta).
    The first step is fused straight from x (LIF_X op); the last step
    fuses its spike compare (LIF_SPK op).
  - Spike recovery OFF the serial chain, on the otherwise-idle Scalar
    engine: Sigmoid(1e30 * zeta) = exact saturated 0/1.
  - Pair products on the Vector engine (bf16, 2x mode); triple products
    and some pair groups on GPSIMD (otherwise idle).
  - Time-accumulation on the TensorEngine: per-weight-class SCALED
    identity matmuls (10 alpha*I matrices) accumulate EVERY feature,
    pre-weighted, into ONE shared [128, 3, 32] PSUM region -- the
    epilogue is a single 96-wide reduce; k1/out0 finalize on host.
  - Pipeline: chunk-i recurrence steps interleave chunk-(i-2) product
    ops so the Vector engine never stalls on recovery latency; cell-sum
    matmuls issue as soon as recovery lands, pair matmuls after their
    products.  TimelineSim-tuned chunk bounds.
"""

import numpy as np

B = 32768
N_CORES = 8
B_CORE = B // N_CORES          # 4096
P = 128                        # partitions
SPP = B_CORE // P              # 32 samples per partition
C = 9                          # cells
T = 25                         # timesteps (t=0 spikes are always 0)
NT = T - 1                     # active timesteps t=1..24 (tau index 0..23)
BETA = 0.95

# chunk bounds over tau=0..23 (spike index); recovery+products pipeline
BOUNDS = (4, 8, 14, 17, 20, 22, 23, 24)
N_WARM = 8                     # dummy warm-up matmuls
WARM_FD = 384

_STATE: dict = {}


def _host_coeffs(W1, b1, W2, b2, W3, b3, W4, b4):
    """8-entry LUT of the line-MLP p1 output -> multilinear coeffs.
    Returns (alphas[10], k1): the 10 weight-class scalars for the scaled
    identity matrices, and the constant term."""
    W1, b1, W2, b2, W3, b3, W4, b4 = [
        np.asarray(a, np.float64) for a in (W1, b1, W2, b2, W3, b3, W4, b4)
    ]

    def mlp_p1(s):
        h = np.maximum(W1 @ s + b1, 0)
        h = np.maximum(W2 @ h + b2, 0)
        h = np.maximum(W3 @ h + b3, 0)
        h = np.maximum(W4 @ h + b4, 0)
        e = np.exp(h - h.max())
        return e[1] / e.sum()

    u = np.zeros(8)
    for code in range(8):
        s = np.array([(code >> j) & 1 for j in range(3)], np.float64)
        u[code] = mlp_p1(s)

    # Moebius transform: u(s) = sum_m c[m] * prod_{j in m} s_j
    c = np.zeros(8)
    for m in range(8):
        for mp in range(8):
            if (mp & m) == mp:
                c[m] += (-1) ** bin(m ^ mp).count("1") * u[mp]

    c_s = [c[1], c[2], c[4]]
    # cell weight classes by sorted (i, j) of cell=3i+j; each cell is in
    # row-line i (position j) and col-line j (position i)
    cellw = {
        (0, 0): c_s[0] + c_s[0], (0, 1): c_s[0] + c_s[1],
        (0, 2): c_s[0] + c_s[2], (1, 1): c_s[1] + c_s[1],
        (1, 2): c_s[1] + c_s[2], (2, 2): c_s[2] + c_s[2],
    }
    alphas = [cellw[(0, 0)], cellw[(0, 1)], cellw[(0, 2)], cellw[(1, 1)],
              cellw[(1, 2)], cellw[(2, 2)], c[3], c[6], c[5], c[7]]
    k1 = 150.0 * c[0]
    return np.array(alphas), k1


def _register_aux_ops():
    """LIF_X_ANT: first step straight from x:
         out = s0*in0 + s1 - (in0 > imm2)    [in1 unused]
       LIF_SPK_ANT: fused last step + spike compare:
         out = (s0*in0 + in1 - (in0 > 0)) > 0
    """
    import re
    from concourse import dve_ops
    from concourse.dve_spec import Spec, Src0, Src1, C0, C1, C2, Zero

    inner = Src0 * C0 + Src1 - (Src0 > Zero)
    out = []
    for name, spec in (
        ("LIF_X_ANT", Spec(
            body=Src0 * C0 + C1 - (Src0 > C2),
            reference=lambda in0, in1, s0, s1, imm2: in0 * s0 + s1
            - (in0 > imm2).astype(in0.dtype))),
        ("LIF_SPK_ANT", Spec(
            body=(Src0 * C0 + Src1 - (Src0 > Zero)) > Zero,
            reference=lambda in0, in1, s0, s1, imm2: (
                (in0 * s0 + in1 - (in0 > 0).astype(in0.dtype)) > 0
            ).astype(in0.dtype))),
        ("LIF_ZETA2_ANT", Spec(
            body=inner * C0 + Src1 - (inner > Zero),
            reference=lambda in0, in1, s0, s1, imm2: (
                lambda z1: z1 * s0 + in1 - (z1 > 0).astype(in0.dtype))(
                in0 * s0 + in1 - (in0 > 0).astype(in0.dtype)))),
    ):
        existing = [o for o in dve_ops.OPS if o.name == name]
        if existing:
            out.append(existing[0])
            continue
        op = dve_ops.DveOp(name, spec, subdim=False, uops_sha={})
        dve_ops.OPS.append(op)
        dve_ops.CUSTOM_DVE_SPECS[op.name] = spec
        dve_ops._SUB_OPCODE_FOR_NAME[op.name] = (
            max(dve_ops._SUB_OPCODE_FOR_NAME.values()) + 1)
        for ver in ("v3", "v4"):
            try:
                op.compile(ver)
            except ValueError as e:
                m = re.search(r'\]="([0-9a-f]+)"', str(e))
                if not m:
                    raise
                op.uops_sha[ver] = m.group(1)
        out.append(op)
    return out


def _register_zeta_op():
    """Custom fused DVE op: out = s0*in0 + in1 - (in0 > 0)  (LIF step in
    zeta-space). Self-pins the uops sha; numerics verified end-to-end."""
    import re
    from concourse import dve_ops
    from concourse.dve_spec import Spec, Src0, Src1, C0, Zero

    for o in dve_ops.OPS:
        if o.name == "LIF_ZETA_ANT":
            return o
    spec = Spec(
        body=Src0 * C0 + Src1 - (Src0 > Zero),
        reference=lambda in0, in1, s0, s1, imm2: in0 * s0 + in1
        - (in0 > 0).astype(in0.dtype),
    )
    op = dve_ops.DveOp("LIF_ZETA_ANT", spec, subdim=False, uops_sha={})
    dve_ops.OPS.append(op)
    dve_ops.CUSTOM_DVE_SPECS[op.name] = spec
    dve_ops._SUB_OPCODE_FOR_NAME[op.name] = (
        max(dve_ops._SUB_OPCODE_FOR_NAME.values()) + 1)
    for ver in ("v3", "v4"):
        try:
            op.compile(ver)
        except ValueError as e:
            m = re.search(r'\]="([0-9a-f]+)"', str(e))
            if not m:
                raise
            op.uops_sha[ver] = m.group(1)
    return op


def _build_module(bounds=BOUNDS, n_warm=N_WARM, y_queue="sp",
                  pool_pairs_chunks=1, late_psb_first=False,
                  x_queue="sp", pool_rp_chunks=0, pbounds=None,
                  prologue="full", pool_tri_tail=2, serial_products=0,
                  pool_fill_chunks=0):
    import concourse.bass as bass
    import concourse.tile as tile
    from concourse import bacc, mybir
    from contextlib import ExitStack

    zeta_op = _register_zeta_op()
    x_op, spk_op, d_op = _register_aux_ops()

    f32 = mybir.dt.float32
    bf16 = mybir.dt.bfloat16
    Alu = mybir.AluOpType
    Act = mybir.ActivationFunctionType

    nc = bacc.Bacc("TRN2", target_bir_lowering=False, debug=False,
                   num_devices=N_CORES)

    # blob per partition: [ 10 scaled identities (128 bf16 = 64 f32 each)
    #                     | k1 | 150-k1 ]
    NID = 10
    BLOB = NID * (P // 2) + 4
    xs = nc.declare_dram_parameter("xs", [P, C, SPP], f32, isOutput=False)
    blob = nc.declare_dram_parameter("blob", [P, BLOB], f32, isOutput=False)
    y = nc.declare_dram_parameter("y", [B_CORE], f32, isOutput=True)

    H = SPP // 2

    with tile.TileContext(nc) as tc, ExitStack() as ctx:
        pool = ctx.enter_context(tc.tile_pool(name="main", bufs=1))
        psum = ctx.enter_context(tc.tile_pool(name="psum", bufs=1, space="PSUM"))

        # ---- ACT table warm-up: dummy sigmoid so the LoadActFuncSet
        # (~1.3us) runs before x arrives instead of delaying recovery ----
        act_warm = pool.tile([P, 2], f32)
        nc.gpsimd.memset(act_warm, 0)
        nc.scalar.activation(act_warm, act_warm, Act.Sigmoid, scale=1.0)

        # ---- input DMAs: host ships tau = x - (1-beta), pre-permuted
        # to [P, C, SPP] (bit-identical f32 subtract done host-side) ----
        tau = pool.tile([P, C, SPP], f32)
        nc.sync.dma_start(tau, xs[:, :, :])
        blob_sb = pool.tile([P, BLOB], f32)
        nc.sync.dma_start(blob_sb, blob[:, :])
        ids = blob_sb[:, :NID * (P // 2)].bitcast(bf16).rearrange(
            "p (k q) -> p k q", k=NID)          # [P, 10, 128]
        k1_ap = blob_sb[:, NID * (P // 2):NID * (P // 2) + 1]
        k150_ap = blob_sb[:, NID * (P // 2) + 1:NID * (P // 2) + 2]
        nbeta_ap = blob_sb[:, NID * (P // 2) + 2:NID * (P // 2) + 3]
        nbig_ap = blob_sb[:, NID * (P // 2) + 3:NID * (P // 2) + 4]

        # zh[k] = zeta_{k+2}, k = 0..23; zeta_1 = x-1 never spikes.
        # tau arrives pre-computed and pre-permuted from the host.
        zh = pool.tile([P, NT, C, SPP], f32)

        # spikes sh[tau] = (zeta_{tau+2} > 0), tau = 0..23   (bf16 0/1)
        sh = pool.tile([P, NT, C, SPP], bf16)
        sh_r = sh.rearrange("p t (i j) s -> p t i j s", i=3)

        # product history (bf16)
        rp01 = pool.tile([P, NT, 3, 2, SPP], bf16)
        rp02 = pool.tile([P, NT, 3, SPP], bf16)
        rtr = pool.tile([P, NT, 3, SPP], bf16)
        cp03 = pool.tile([P, NT, 6, SPP], bf16)
        cp06 = pool.tile([P, NT, 3, SPP], bf16)
        ctr = pool.tile([P, NT, 3, SPP], bf16)

        # Single shared PSUM accumulator: every (pre-weighted) matmul
        # accumulates into the same [P, 3, SPP] region -- the final
        # feature reduce is then only 3*SPP wide.
        ps_all = psum.tile([P, 3, SPP], f32)

        # identity index per cell (cell c = 3i+j -> class {i, j})
        IA00, IA01, IA02, IA11, IA12, IA22, IC3, IC6, IC5, IC7 = range(NID)
        CELL_ID = [IA00, IA01, IA02, IA01, IA11, IA12, IA02, IA12, IA22]

        assert bounds[-1] == NT
        nchunks = len(bounds)
        chunks = [(0 if ci == 0 else bounds[ci - 1], b)
                  for ci, b in enumerate(bounds)]

        def recovery(t0, t1, engine):
            if t0 == 0:
                # sh[0] = ((1+beta)x - 1 > 0) with x = tau + (1-beta)
                nc.scalar.activation(sh[:, 0], tau, Act.Sigmoid,
                                     scale=(1.0 + BETA) * 1e30, bias=nbig_ap)
                t0 = 1
            t1 = min(t1, NT - 1)   # sh[NT-1] comes from the fused step
            if t1 <= t0:
                return
            if engine == "act":
                nc.scalar.activation(sh[:, t0:t1], zh[:, t0:t1],
                                     Act.Sigmoid, scale=1e30)
            else:
                nc.vector.tensor_scalar(out=sh[:, t0:t1], in0=zh[:, t0:t1],
                                        scalar1=0.0, scalar2=None,
                                        op0=Alu.is_gt)

        def product_thunks(t0, t1, pool_triples, pool_pairs=False,
                           pool_rp=False):
            """Deferred product ops for chunk [t0, t1). Triple products go
            to GPSIMD for early chunks (otherwise idle); the last chunks
            keep them on DVE so the PE/epilogue isn't gated on a lagging
            Pool queue."""
            tsl = slice(t0, t1)
            tri = nc.gpsimd if pool_triples else nc.vector
            pr = nc.gpsimd if pool_pairs else nc.vector

            def _rp01():
                nc.vector.tensor_mul(rp01[:, tsl], sh_r[:, tsl, :, 0:2],
                                     sh_r[:, tsl, :, 1:3])
                tri.tensor_mul(rtr[:, tsl], rp01[:, tsl, :, 0],
                               sh_r[:, tsl, :, 2])

            def _cp03():
                nc.vector.tensor_mul(cp03[:, tsl], sh[:, tsl, 0:6],
                                     sh[:, tsl, 3:9])
                tri.tensor_mul(ctr[:, tsl], cp03[:, tsl, 0:3],
                               sh[:, tsl, 6:9])

            pair_thunks = [
                lambda: pr.tensor_mul(rp02[:, tsl],
                                      sh_r[:, tsl, :, 0],
                                      sh_r[:, tsl, :, 2]),
                lambda: pr.tensor_mul(cp06[:, tsl], sh[:, tsl, 0:3],
                                      sh[:, tsl, 6:9]),
            ]
            if pool_pairs:
                # pool queue: pairs (spike-only deps) before triples, so the
                # Pool engine isn't head-blocked on DVE-produced rp01/cp03
                return pair_thunks + [_rp01, _cp03]
            return [_rp01, _cp03] + pair_thunks

        def mms_cells(t0, t1):
            """cell-sum matmuls: depend only on recovery (not products),
            so they're emitted right after the chunk's recovery. The very
            first one carries start=True for the shared psum group."""
            for t in range(t0, t1):
                for c in range(C):
                    nc.tensor.matmul(ps_all[:, 0:1], ids[:, CELL_ID[c]],
                                     sh[:, t, c:c + 1],
                                     start=(t == 0 and c == 0), stop=False,
                                     skip_group_check=True)

        def mms_pairs(t0, t1):
            for t in range(t0, t1):
                sp = t == NT - 1
                items = [(IC3, rp01[:, t, :, 0]),
                         (IC6, rp01[:, t, :, 1]),
                         (IC5, rp02[:, t]),
                         (IC7, rtr[:, t]),
                         (IC3, cp03[:, t, 0:3]),
                         (IC6, cp03[:, t, 3:6]),
                         (IC5, cp06[:, t]),
                         (IC7, ctr[:, t])]
                for i, (idk, rhs) in enumerate(items):
                    nc.tensor.matmul(ps_all[:, 0:3], ids[:, idk], rhs,
                                     start=False,
                                     stop=sp and i == len(items) - 1,
                                     skip_group_check=True)

        # ---- main pipeline ----
        # DVE stream: chunk-i steps interleave chunk-(i-2) product ops
        # (2-chunk lag hides the ACT recovery latency). First two chunks
        # have no partner work: emit steps as halves to hide pipe drain.
        def chain_step(k):
            """advance the serial chain along ODD zh indices (2 LIF steps
            per fused op); k == NT-1 fuses the final spike compare."""
            if k == 1:
                # zeta_3 as a pure function of tau = x - (1-beta):
                # zeta_3 = s0*x - 1 - (x > 1/(1+beta)), x = tau + (1-beta)
                s0x = BETA * (1.0 + BETA) + 1.0
                nc.vector._custom_dve(
                    x_op, out=zh[:, 1], in0=tau,
                    s0=s0x,
                    s1=(1.0 - BETA) * s0x - 1.0,
                    imm2=1.0 / (1.0 + BETA) - (1.0 - BETA))
            elif k == NT - 1:
                nc.vector._custom_dve(spk_op, out=sh[:, NT - 1],
                                      in0=zh[:, k - 1], in1=tau, s0=BETA)
            else:
                nc.vector._custom_dve(d_op, out=zh[:, k], in0=zh[:, k - 2],
                                      in1=tau, s0=BETA)

        fill_scratch = pool.tile([P, 2, C, SPP], f32)

        def even_fill_pool(evens):
            """same reconstruction via 3 stock GPSIMD ops -- runs in the
            Pool engine's otherwise-dead start window."""
            e0, e1 = evens[0], evens[-1]
            n = len(evens)
            g = fill_scratch[:, 0:n] if n <= 2 else None
            m = fill_scratch[:, 0:n]
            src_ap = zh[:, e0 - 1:e1:2]
            nc.gpsimd.tensor_scalar(out=fill_scratch[:, 0:n], in0=src_ap,
                                    scalar1=0.0, scalar2=None, op0=Alu.is_gt)
            nc.gpsimd.scalar_tensor_tensor(
                out=zh[:, e0:e1 + 1:2], in0=src_ap, scalar=BETA,
                in1=fill_scratch[:, 0:n], op0=Alu.mult, op1=Alu.subtract)
            tau_b = tau.rearrange("p (o c) s -> p o c s", o=1).broadcast_to(
                (P, n, C, SPP))
            nc.gpsimd.tensor_tensor(out=zh[:, e0:e1 + 1:2],
                                    in0=zh[:, e0:e1 + 1:2], in1=tau_b,
                                    op=Alu.add)

        def even_fill(evens):
            """reconstruct even zh states from the odd chain, batched."""
            if not evens:
                return
            e0, e1 = evens[0], evens[-1]
            n = len(evens)
            tau_b = tau.rearrange("p (o c) s -> p o (c s)", o=1).broadcast_to(
                (P, n, C * SPP))
            nc.vector._custom_dve(
                zeta_op,
                out=zh[:, e0:e1 + 1:2].rearrange("p t c s -> p t (c s)"),
                in0=zh[:, e0 - 1:e1:2].rearrange("p t c s -> p t (c s)"),
                in1=tau_b, s0=BETA)

        # Product ranges may merge several recovery chunks (fewer, larger
        # DVE ops -> fewer fixed-cost bubbles). Each range becomes ready
        # when its covering recovery chunks have been emitted; its ops are
        # interleaved into steps two chunks later.
        from collections import deque
        pb = list(pbounds) if pbounds is not None else list(bounds)
        assert pb[-1] == NT and set(pb) <= set(bounds)
        pranges = [(0 if i == 0 else pb[i - 1], b) for i, b in enumerate(pb)]
        pending: deque = deque()   # (ready_chunk_idx, thunk)

        if serial_products:
            # Front-loaded chain: emit ALL recurrence work first (chain ops
            # back-to-back with even-fills absorbing the pipe drains), then
            # the whole product phase as pure backlog.  The serial chain
            # finishes ~8us earlier; products become busy-bound.
            for ci, (t0, t1) in enumerate(chunks):
                chain_ks = [k for k in range(max(t0, 1), t1) if k % 2 == 1]
                evens = [k for k in range(max(t0, 2), min(t1, NT - 1))
                         if k % 2 == 0]
                for j, k in enumerate(chain_ks):
                    chain_step(k)
                    if j == 0:
                        even_fill(evens)
                if not chain_ks:
                    even_fill(evens)
                recovery(t0, t1, "act")
                mms_cells(t0, t1)
            for ri, (r0, r1) in enumerate(pranges):
                for th in product_thunks(
                        r0, r1, pool_triples=(ri < serial_products),
                        pool_pairs=(ri < pool_pairs_chunks),
                        pool_rp=(ri < pool_rp_chunks)):
                    th()
                mms_pairs(r0, r1)
        else:
            for ci, (t0, t1) in enumerate(chunks):
                chain_ks = [k for k in range(max(t0, 1), t1) if k % 2 == 1]
                evens = [k for k in range(max(t0, 2), min(t1, NT - 1))
                         if k % 2 == 0]
                fill = (even_fill_pool if ci < pool_fill_chunks
                        else even_fill)
                for j, k in enumerate(chain_ks):
                    chain_step(k)
                    if j == 0:
                        fill(evens)
                    if pending and pending[0][0] <= ci - 2:
                        pending.popleft()[1]()
                if not chain_ks:
                    fill(evens)
                while pending and pending[0][0] <= ci - 2:
                    pending.popleft()[1]()
                recovery(t0, t1, "dve" if ci == nchunks - 1 else "act")
                mms_cells(t0, t1)
                for ri, (r0, r1) in enumerate(pranges):
                    if r1 == t1:
                        for th in product_thunks(
                                r0, r1,
                                pool_triples=(ri < len(pranges)
                                              - pool_tri_tail),
                                pool_pairs=(ri < pool_pairs_chunks),
                                pool_rp=(ri < pool_rp_chunks)):
                            pending.append((ci, th))
                        pending.append((ci, lambda r0=r0, r1=r1:
                                        mms_pairs(r0, r1)))
            while pending:
                pending.popleft()[1]()

        # ---- epilogue: one tiny reduce; k1/out0 finalized on host ----
        red = pool.tile([P, SPP], f32)
        nc.vector.tensor_reduce(out=red,
                                in_=ps_all.rearrange("p f s -> p s f"),
                                axis=mybir.AxisListType.X, op=Alu.add)
        yq = nc.gpsimd if y_queue == "pool" else nc.sync
        yq.dma_start(y.rearrange("(p s) -> p s", p=P), red)

    nc.compile()
    return nc


def _get_module():
    if "nc" not in _STATE:
        _STATE["nc"] = _build_module()
    return _STATE["nc"]


def kernel(x, W1, b1, W2, b2, W3, b3, W4, b4, _trace=False):
    import ml_dtypes
    from concourse.bass_utils import run_bass_kernel_spmd

    alphas, k1 = _host_coeffs(W1, b1, W2, b2, W3, b3, W4, b4)

    xs = np.asarray(x, np.float32).reshape(N_CORES, P, SPP, C)
    xs = np.ascontiguousarray(
        (xs - np.float32(1.0 - BETA)).transpose(0, 1, 3, 2))
    eye = np.eye(P, dtype=np.float64)
    ids = np.concatenate(
        [np.ascontiguousarray((eye * a).astype(ml_dtypes.bfloat16)).view(
            np.float32) for a in alphas], axis=1)          # [P, 10*64]
    consts = np.tile(
        np.array([[k1, 150.0 - k1, -(1.0 - BETA), -BETA * BETA * 1e30]],
                 np.float32), (P, 1))
    blob = np.ascontiguousarray(
        np.concatenate([ids.astype(np.float32), consts], axis=1))

    nc = _get_module()
    in_maps = [{"xs": xs[i], "blob": blob} for i in range(N_CORES)]
    res = run_bass_kernel_spmd(nc, in_maps, core_ids=list(range(N_CORES)),
                               trace=_trace)
    red = np.concatenate([res.results[i]["y"] for i in range(N_CORES)])
    out1 = (red.astype(np.float64) + k1).astype(np.float32)
    out = np.stack([150.0 - out1, out1], axis=1).astype(np.float32)
    if _trace:
        _STATE["last_results"] = res
    return out
